# revision 4
# baseline (speedup 1.0000x reference)
"""SSIM-based loss kernel for Trainium2 (8 NeuronCores, data-parallel over batch).

Computes: loss = 1 - (1 + mean(SSIM(sigmoid(seg), sigmoid(edge)))) / 2
for seg, edge of shape [32, 1, 512, 512] fp32, SSIM with a 7x7 gaussian
window (sigma=1.5), SAME zero-padding, C1=0.01^2, C2=0.03^2.

Sharding: batch dim across 8 cores (4 images each). Each core returns a
[1,1] partial sum of its ssim map; the host reduces and forms the scalar.

Per-core algorithm (separable blur on the tensor engine, bf16 data path):
  s = sigmoid(seg), e = sigmoid(edge)
  maps: P = s+e, M = s-e, W = s*e, Q = M^2 + 2W = s^2+e^2
  blur pipes (7x7 gaussian = two 1D banded matmuls), scales folded into the
  step-2 band variants:
    A* = blur(P)/sqrt(2), B* = blur(M)/sqrt(2)
    U* = blur(Q)       (= sigma1_sq + sigma2_sq + mu1^2 + mu2^2)
    V* = 2*blur(W)     (= 2*sigma12 + 2*mu1*mu2)
  x = A*^2, y = B*^2
  alpha = x - y + C1 = 2 mu1 mu2 + C1
  beta  = x + y + C1 = mu1^2 + mu2^2 + C1
  gamma = V* - alpha + C1 + C2 = 2 sigma12 + C2
  delta = U* - beta + C1 + C2 = sigma1^2 + sigma2^2 + C2
  ssim  = (alpha*gamma) / (beta*delta)

Images live in SBUF as 5 overlapping 128-row "halo" chunks (rows R[c]..R[c]+128)
so each 1D blur output region O[c]..O[c+1] is produced by a single matmul with
no cross-chunk accumulation. Step 1 uses the image chunk as the stationary
operand (output comes out transposed, already in halo layout along w); step 2
uses the band matrix as stationary (one matmul per output region).

Engine balance:
  - all matmuls bf16 (1 cyc/row vs 4 for fp32); tolerance is 2e-2, measured
    error ~7e-4, dominated by bf16 map/z rounding
  - scalar: sigmoid, z evacuation (PSUM->SBUF bf16), x/y squares of PSUM
  - gpsimd: W, M^2, Q map building (SBUF-only bf16)
  - vector: P/M, per-tile alpha/beta/gamma/delta STTs, batched per-image
    nu/dn/recip/jk tail
  - tensor: blurs + per-tile ssim-sum via ones-vector matmul accumulating
    into one PSUM [1,512] across the whole kernel
"""

import numpy as np
import ml_dtypes

import concourse.bass as bass
import concourse.bacc as bacc
import concourse.tile as tile
import concourse.mybir as mybir
from concourse.bass_utils import run_bass_kernel_spmd

WS = 7
HW = WS // 2
SIGMA = 1.5
C1 = 0.01 ** 2
C2 = 0.03 ** 2

N_CORES = 8
IMG = 512
P = 128
PER_CORE = 4

# halo chunking: out regions [O[c], O[c+1]), input rows [R[c], R[c]+128)
O = [0, 122, 244, 366, 488, 512]
R = [0, 119, 241, 363, 384]
NC5 = 5
FD5 = NC5 * IMG  # 2560

F32 = mybir.dt.float32
BF16 = mybir.dt.bfloat16
NPBF = np.dtype(ml_dtypes.bfloat16)
AF = mybir.ActivationFunctionType
OP = mybir.AluOpType


def _gauss():
    x = np.arange(WS, dtype=np.float64)
    g = np.exp(-((x - HW) ** 2) / (2.0 * SIGMA ** 2))
    return g / g.sum()


def _band_tiles(scale):
    """B_c[r, j] = g[(O[c]+j) - (R[c]+r)] for tap offsets in [-3,3], zero
    otherwise. Serves as step-1 moving operand and step-2 stationary."""
    g = _gauss() * scale
    tiles = []
    for c in range(NC5):
        w = O[c + 1] - O[c]
        t = np.zeros((P, w), dtype=np.float64)
        for r in range(P):
            i = R[c] + r
            for j in range(w):
                d = (O[c] + j) - i
                if -HW <= d <= HW:
                    t[r, j] = g[d + HW]
        tiles.append(t.astype(np.float32))
    return tiles


_CACHE = {}


def _build():
    if "nc" in _CACHE:
        return _CACHE["nc"]

    nc = bacc.Bacc(None)

    seg_d = nc.dram_tensor("seg", [PER_CORE, IMG, IMG], F32, kind="ExternalInput")
    edge_d = nc.dram_tensor("edge", [PER_CORE, IMG, IMG], F32, kind="ExternalInput")
    out_d = nc.dram_tensor("out", [1, 1], F32, kind="ExternalOutput")

    # Band variants: 0: step1 + U pipe (scale 1); 1: mu pipes (1/sqrt2);
    # 2: V pipe (scale 2)
    variants = [1.0, 1.0 / np.sqrt(2.0), 2.0]
    packed, offsets = [], []
    col = 0
    for v in variants:
        offs = []
        for t in _band_tiles(v):
            offs.append((col, t.shape[1]))
            packed.append(t)
            col += t.shape[1]
        offsets.append(offs)
    band_np = np.concatenate(packed, axis=1).astype(NPBF)  # [128, 1536] bf16
    band_d = nc.inline_tensor(band_np, name="band")
    ones_d = nc.inline_tensor(np.ones((P, 1), dtype=NPBF), name="ones")

    K12 = float(C1 + C2)

    with tile.TileContext(nc) as tc:
        with (
            tc.tile_pool(name="const", bufs=1) as constp,
            tc.tile_pool(name="io", bufs=2) as iop,
            tc.tile_pool(name="sig", bufs=1) as sigp,
            tc.tile_pool(name="maps", bufs=1) as mapp,
            tc.tile_pool(name="zmaps", bufs=2) as zp,
            tc.tile_pool(name="post", bufs=1) as postp,
            tc.tile_pool(name="acc", bufs=1) as accp,
            tc.tile_pool(name="psz", bufs=2, space="PSUM") as psz,
            tc.tile_pool(name="ps2", bufs=1, space="PSUM") as ps2,
            tc.tile_pool(name="psacc", bufs=1, space="PSUM") as psacc,
        ):
            band = constp.tile([P, band_np.shape[1]], BF16)
            nc.sync.dma_start(band[:], band_d[:])
            ones = constp.tile([P, 1], BF16)
            nc.sync.dma_start(ones[:], ones_d[:])

            def band_ap(v, c):
                c0, w = offsets[v][c]
                return band[:, c0:c0 + w], w

            pacc = psacc.tile([1, IMG], F32)
            n_acc = PER_CORE * NC5
            i_acc = 0

            for b in range(PER_CORE):
                sg = iop.tile([P, NC5, IMG], F32, tag="sg")
                ed = iop.tile([P, NC5, IMG], F32, tag="ed")
                for c in range(NC5):
                    nc.sync.dma_start(sg[:, c, :], seg_d[b, R[c]:R[c] + P, :])
                    nc.sync.dma_start(ed[:, c, :], edge_d[b, R[c]:R[c] + P, :])

                sgb = sigp.tile([P, NC5, IMG], BF16, tag="sgb")
                edb = sigp.tile([P, NC5, IMG], BF16, tag="edb")
                nc.scalar.activation(sgb[:], sg[:], AF.Sigmoid)
                nc.scalar.activation(edb[:], ed[:], AF.Sigmoid)

                sf = sgb[:].rearrange("p c w -> p (c w)")
                ef = edb[:].rearrange("p c w -> p (c w)")
                Pt = mapp.tile([P, NC5, IMG], BF16, tag="P")
                Mt = mapp.tile([P, NC5, IMG], BF16, tag="M")
                Wt = mapp.tile([P, NC5, IMG], BF16, tag="W")
                S1t = mapp.tile([P, NC5, IMG], BF16, tag="S1")
                S2t = mapp.tile([P, NC5, IMG], BF16, tag="S2")
                Qt = mapp.tile([P, NC5, IMG], BF16, tag="Q")
                flat = lambda t: t[:].rearrange("p c w -> p (c w)")
                nc.vector.tensor_tensor(flat(Pt), sf, ef, OP.add)
                nc.vector.tensor_tensor(flat(Mt), sf, ef, OP.subtract)
                nc.vector.tensor_tensor(flat(Wt), sf, ef, OP.mult)
                nc.gpsimd.tensor_tensor(flat(S1t), sf, sf, OP.mult)
                nc.gpsimd.tensor_tensor(flat(S2t), ef, ef, OP.mult)
                nc.gpsimd.tensor_tensor(flat(Qt), flat(S1t), flat(S2t), OP.add)

                # ---- blur step 1: Z[w, ho] (transposed, halo layout along w)
                srcs = {"zP": Pt, "zM": Mt, "zQ": Qt, "zW": Wt}
                zt = {}
                for name, src in srcs.items():
                    z = zp.tile([P, NC5, IMG], BF16, tag=name)
                    zt[name] = z
                    for k in range(NC5):
                        pz = psz.tile([P, IMG], F32, tag="pz")
                        for c in range(NC5):
                            rhs, w = band_ap(0, c)
                            nc.tensor.matmul(
                                pz[:, O[c]:O[c + 1]],
                                src[:, c, R[k]:R[k] + P],
                                rhs,
                                start=(c == 0),
                                stop=(c == NC5 - 1),
                            )
                        nc.scalar.copy(z[:, k, :], pz[:])

                # ---- blur step 2 + per-tile pointwise; batched tail below.
                xp = postp.tile([P, NC5, IMG], BF16, tag="xp")
                yp = postp.tile([P, NC5, IMG], BF16, tag="yp")
                al = postp.tile([P, NC5, IMG], BF16, tag="al")
                be = postp.tile([P, NC5, IMG], BF16, tag="be")
                ga = postp.tile([P, NC5, IMG], BF16, tag="ga")
                de = postp.tile([P, NC5, IMG], BF16, tag="de")
                for k in range(NC5):
                    wk = O[k + 1] - O[k]
                    pa = ps2.tile([P, IMG], F32, tag="pa")
                    pb = ps2.tile([P, IMG], F32, tag="pb")
                    pu = ps2.tile([P, IMG], F32, tag="pu")
                    pv = ps2.tile([P, IMG], F32, tag="pv")
                    bmu, _ = band_ap(1, k)
                    b1, _ = band_ap(0, k)
                    b2, _ = band_ap(2, k)
                    nc.tensor.matmul(pa[:wk, :], bmu, zt["zP"][:, k, :], start=True, stop=True)
                    nc.tensor.matmul(pb[:wk, :], bmu, zt["zM"][:, k, :], start=True, stop=True)
                    nc.tensor.matmul(pu[:wk, :], b1, zt["zQ"][:, k, :], start=True, stop=True)
                    nc.tensor.matmul(pv[:wk, :], b2, zt["zW"][:, k, :], start=True, stop=True)

                    nc.scalar.activation(xp[:wk, k, :], pa[:wk, :], AF.Square)
                    nc.scalar.activation(yp[:wk, k, :], pb[:wk, :], AF.Square)
                    nc.vector.scalar_tensor_tensor(
                        al[:wk, k, :], xp[:wk, k, :], C1, yp[:wk, k, :], OP.add, OP.subtract)
                    nc.vector.scalar_tensor_tensor(
                        be[:wk, k, :], xp[:wk, k, :], C1, yp[:wk, k, :], OP.add, OP.add)
                    nc.vector.scalar_tensor_tensor(
                        ga[:wk, k, :], pv[:wk, :], K12, al[:wk, k, :], OP.add, OP.subtract)
                    nc.vector.scalar_tensor_tensor(
                        de[:wk, k, :], pu[:wk, :], K12, be[:wk, k, :], OP.add, OP.subtract)

                # ---- batched per-image tail on [128, 2560] ----
                nu = postp.tile([P, NC5, IMG], BF16, tag="nu")
                dn = postp.tile([P, NC5, IMG], F32, tag="dn")
                nc.vector.tensor_tensor(flat(nu), flat(al), flat(ga), OP.mult)
                nc.vector.tensor_tensor(flat(dn), flat(be), flat(de), OP.mult)
                rc = postp.tile([P, NC5, IMG], F32, tag="rc")
                nc.vector.reciprocal_approx_fast(flat(rc), flat(dn))
                jk = postp.tile([P, NC5, IMG], BF16, tag="jk")
                nc.vector.tensor_tensor(flat(jk), flat(nu), flat(rc), OP.mult)

                for k in range(NC5):
                    wk = O[k + 1] - O[k]
                    nc.tensor.matmul(
                        pacc[:, :],
                        ones[:wk, :],
                        jk[:wk, k, :],
                        start=(i_acc == 0),
                        stop=(i_acc == n_acc - 1),
                        skip_group_check=True,
                    )
                    i_acc += 1

            accs = accp.tile([1, IMG], F32)
            nc.scalar.copy(accs[:], pacc[:])
            final = accp.tile([1, 1], F32)
            nc.vector.tensor_reduce(final[:], accs[:], mybir.AxisListType.X, OP.add)
            nc.sync.dma_start(out_d[:], final[:])

    nc.compile()
    _CACHE["nc"] = nc
    return nc


def kernel(seg: np.ndarray, edge: np.ndarray) -> np.ndarray:
    nc = _build()
    seg = np.ascontiguousarray(seg, dtype=np.float32).reshape(N_CORES, PER_CORE, IMG, IMG)
    edge = np.ascontiguousarray(edge, dtype=np.float32).reshape(N_CORES, PER_CORE, IMG, IMG)
    in_maps = [{"seg": seg[c], "edge": edge[c]} for c in range(N_CORES)]
    res = run_bass_kernel_spmd(nc, in_maps, list(range(N_CORES)))
    total = 0.0
    for c in range(N_CORES):
        total += float(res.results[c]["out"].astype(np.float64).sum())
    mssim = total / (32.0 * IMG * IMG)
    return np.float32(1.0 - (1.0 + mssim) / 2.0)
